# revision 13
# baseline (speedup 1.0000x reference)
"""DISCOBlock Trainium2 kernel.

Sharding: batch (4) x image-row-halves (2) across 8 cores. Each core computes
one half of one batch element's two conv+instancenorm+leakyrelu layers,
including a 5-row halo of layer-1 output it computes redundantly, so the only
cross-core communication is a tiny per-channel statistics AllGather (pairwise)
before each instance norm.

Conv algorithm: row-interleaved planes. The padded image is stored as 4
row-residue planes on 128 SBUF partitions (partition = residue*32 + channel).
A matmul with K=128 then contracts (4 row-taps x 32 channels) at once; the 4
output row-residues go to 4 PSUM column groups (tile_position col tiling) that
execute concurrently on the PE array. Matmuls run in float32r (TF32-like).

The second (odd) core of each pair gets its rows AND its conv kernels flipped
vertically by the host, which makes the device program identical on all cores
(halo always at the bottom).
"""

import numpy as np

import concourse.bass as bass
import concourse.tile as tile
import concourse.mybir as mybir
from concourse import bacc
from concourse.bass_utils import run_bass_kernel_spmd

# ---- problem constants ----
B, C, H, W = 4, 32, 256, 256
CUTOFF = 0.02
NR, NPHI = 6, 7
KSIZE = 1 + (NR - 1) * NPHI  # 36
EPS = 1e-5
SLOPE = 0.2
R, KW = 5, 11

# ---- scheme constants ----
Wp = W + 10           # padded plane width (5 cols of zero pad each side)
NPX = 40              # x4 planes (160 padded rows)
NPH = 38              # h4 planes (152 padded rows)
NPO = 32              # out4 planes (128 own rows)
RHO0 = 16             # first real x row lands at padded row 16
NROWS_IN = 138        # input rows per core (128 own + 5 halo + 5 conv reach)
ST1 = list(range(4, 35, 2)) + [36]   # conv1 supertiles; halo tile (36) last
ST1_STATS = set(range(4, 35, 2))     # supertiles contributing to norm1 stats
ST2 = list(range(4, 35, 2))          # conv2 supertiles (all contribute stats)
NREC = 16                            # stat records per layer
NOWN = NREC * 512                    # own pixels per partition (8192)
F32 = mybir.dt.float32
F32R = mybir.dt.float32r


def _make_psi():
    hy, hx = 1.0 / (H - 1), 1.0 / (W - 1)
    rr = int(np.floor(CUTOFF * (min(H, W) - 1)))
    iy, ix = np.meshgrid(np.arange(-rr, rr + 1), np.arange(-rr, rr + 1), indexing="ij")
    y, x = iy * hy, ix * hx
    r = np.sqrt(x * x + y * y)
    phi = np.mod(np.arctan2(y, x), 2.0 * np.pi)
    inside = (r <= CUTOFF).astype(np.float64)
    dr = CUTOFF / (NR - 1)
    dphi = 2.0 * np.pi / NPHI
    basis = [np.maximum(0.0, 1.0 - r / dr) * inside]
    for j in range(1, NR):
        rad = np.maximum(0.0, 1.0 - np.abs(r - j * dr) / dr) * inside
        for k in range(NPHI):
            d = np.abs(np.mod(phi - k * dphi + np.pi, 2.0 * np.pi) - np.pi)
            ang = np.maximum(0.0, 1.0 - d / dphi)
            basis.append(rad * ang)
    return np.stack(basis).astype(np.float32)  # [36, 11, 11]


def _make_mask():
    hy = hx = 1.0 / (H - 1)
    iy, ix = np.meshgrid(np.arange(-R, R + 1), np.arange(-R, R + 1), indexing="ij")
    r = np.sqrt((iy * hy) ** 2 + (ix * hx) ** 2)
    return r <= CUTOFF


MASK = _make_mask()


def build_slots():
    """Fused slot list [(s, dxk)]: one M=128 matmul per slot covers all 4
    output residues (weight block [128, 128], rhs shared)."""
    slots = []
    for s in range(-2, 3):
        for dxk in range(KW):
            ok = False
            for g in range(4):
                for d in range(4):
                    dy = 4 * s + g - d
                    if -R <= dy <= R and MASK[dy + R, dxk]:
                        ok = True
            if ok:
                slots.append((s, dxk))
    return slots


SLOTS = build_slots()
NSLOTS = len(SLOTS)


def pack_weights(kern):
    """kern [O, I, 11, 11] -> [128, NSLOTS*128].

    Block k holds lhsT[g*32+i, d*32+o] = kern[o, i, (4s+g-d)+5, dxk].
    """
    wp = np.zeros((128, NSLOTS * 128), np.float32)
    for k, (s, dxk) in enumerate(SLOTS):
        for g in range(4):
            for d in range(4):
                dy = 4 * s + g - d
                if -R <= dy <= R and MASK[dy + R, dxk]:
                    wp[g * 32:(g + 1) * 32, k * 128 + d * 32:k * 128 + (d + 1) * 32] = \
                        kern[:, :, dy + R, dxk].T
    return wp


# ------------------------------------------------------------------
# device program
# ------------------------------------------------------------------

def _conv_layer(nc, pools, src4, w_t, recs, st_list, st_stats, dst_write,
                first_real_plane):
    """Emit one conv layer: supertile matmuls + stat records + psum copies.

    Slots whose input plane pair is entirely in the all-zero top pad
    (u0+s+1 < first_real_plane) are skipped.
    dst_write(u0, psum_tile): copies the [128, 512] supertile out of PSUM.
    """
    psum_pool = pools["psum"]
    for u0 in st_list:
        pt = psum_pool.tile([128, 512], F32, tag="cv")
        live = [(k, s, dxk) for k, (s, dxk) in enumerate(SLOTS)
                if u0 + s + 1 >= first_real_plane]
        for j, (k, s, dxk) in enumerate(live):
            u = u0 + s
            nc.tensor.matmul(
                pt[:, :],
                w_t[:, bass.ts(k, 128)],
                src4[:, u:u + 2, dxk:dxk + W],
                start=(j == 0),
                stop=(j == len(live) - 1),
            )
        if u0 in st_stats:
            i = (u0 - 4) // 2
            nc.vector.bn_stats(out=recs[:, i, :], in_=pt[:, :])
        dst_write(u0, pt)


def _stats_to_scale_bias(nc, pools, recs, cc_in, cc_out, eps_t):
    """recs [128,16,6] -> (rstd [128,1], nb [128,1]) with pairwise AllGather."""
    sp = pools["small"]
    mv = sp.tile([128, 2], F32, tag="mv")
    nc.vector.bn_aggr(out=mv, in_=recs)
    trip = sp.tile([128, 3], F32, tag="trip")
    nc.vector.memset(trip[:, 0:1], float(NOWN))
    nc.vector.tensor_copy(out=trip[:, 1:2], in_=mv[:, 0:1])
    nc.scalar.mul(out=trip[:, 2:3], in_=mv[:, 1:2], mul=float(NOWN))
    nc.sync.dma_start(out=cc_in[:, :], in_=trip)
    nc.gpsimd.collective_compute(
        "AllGather",
        mybir.AluOpType.bypass,
        replica_groups=[[0, 1], [2, 3], [4, 5], [6, 7]],
        ins=[cc_in[:, :]],
        outs=[cc_out[:, :]],
    )
    comb = sp.tile([128, 8, 3], F32, tag="comb")
    # comb[d*32+c, (r, d'), :] = cc_out[r*128 + d'*32 + c, :]
    cc_ap = cc_out[:, :]
    for d in range(4):
        src = bass.AP(
            tensor=cc_ap.tensor,
            offset=0,
            ap=[[3, 32], [384, 2], [96, 4], [1, 3]],
        )
        nc.sync.dma_start(out=comb[d * 32:(d + 1) * 32, :, :], in_=src)
    mvg = sp.tile([128, 2], F32, tag="mvg")
    nc.vector.bn_aggr(out=mvg, in_=comb)
    std = sp.tile([128, 1], F32, tag="std")
    nc.scalar.activation(out=std, in_=mvg[:, 1:2],
                         func=mybir.ActivationFunctionType.Sqrt,
                         bias=eps_t, scale=1.0)
    rstd = sp.tile([128, 1], F32, tag="rstd")
    nc.vector.reciprocal(out=rstd, in_=std)
    negmean = sp.tile([128, 1], F32, tag="negmean")
    nc.scalar.mul(out=negmean, in_=mvg[:, 0:1], mul=-1.0)
    nb = sp.tile([128, 1], F32, tag="nb")
    nc.vector.tensor_mul(out=nb, in0=negmean, in1=rstd)
    return rstd, nb


def build_nc():
    nc = bacc.Bacc(target_bir_lowering=False)
    xin = nc.dram_tensor("xin", [128, NPX, W], F32R, kind="ExternalInput")
    w1 = nc.dram_tensor("w1", [128, NSLOTS * 128], F32R, kind="ExternalInput")
    w2 = nc.dram_tensor("w2", [128, NSLOTS * 128], F32R, kind="ExternalInput")
    out = nc.dram_tensor("out", [128, NPO, W], F32, kind="ExternalOutput")

    cc1_in = nc.dram_tensor("cc1_in", [128, 3], F32)
    cc1_out = nc.dram_tensor("cc1_out", [256, 3], F32)
    cc2_in = nc.dram_tensor("cc2_in", [128, 3], F32)
    cc2_out = nc.dram_tensor("cc2_out", [256, 3], F32)

    with tile.TileContext(nc) as tc:
        with (
            tc.tile_pool(name="big", bufs=1) as big,
            tc.tile_pool(name="small", bufs=2) as small,
            tc.tile_pool(name="psum", bufs=8, space="PSUM") as psum,
        ):
            pools = {"psum": psum, "small": small}

            x4 = big.tile([128, NPX, Wp], F32R)
            h4 = big.tile([128, NPH, Wp], F32R)
            o4 = big.tile([128, NPO, W], F32)
            w1t = big.tile([128, NSLOTS * 128], F32R)
            w2t = big.tile([128, NSLOTS * 128], F32R)
            recs1 = big.tile([128, NREC, 6], F32)
            recs2 = big.tile([128, NREC, 6], F32)
            eps_t = big.tile([128, 1], F32)
            nc.vector.memset(eps_t, EPS)

            # x4 pad cols zero; input DMA (chunked so conv can start early)
            nc.gpsimd.memset(x4[:, :, :].rearrange("p a b -> p (a b)").bitcast(F32), 0.0)
            for lo in range(0, NPX, 8):
                hi = min(lo + 8, NPX)
                nc.sync.dma_start(out=x4[:, lo:hi, 5:261], in_=xin[:, lo:hi, :])
            nc.sync.dma_start(out=w1t, in_=w1[:, :])
            nc.sync.dma_start(out=w2t, in_=w2[:, :])

            # h4 zero pads (planes 0..3 + col strips): full-tile memset
            nc.gpsimd.memset(h4[:, :, :].rearrange("p a b -> p (a b)").bitcast(F32), 0.0)

            # ---- layer 1 ----
            def write1(u0, pt):
                nc.scalar.copy(
                    out=h4[:, u0:u0 + 2, 5:261],
                    in_=pt[:, :].rearrange("p (a b) -> p a b", a=2),
                )

            _conv_layer(nc, pools, x4, w1t, recs1, ST1, ST1_STATS, write1, 4)
            rstd1, nb1 = _stats_to_scale_bias(nc, pools, recs1, cc1_in, cc1_out, eps_t)

            # normalize + leakyrelu h4 planes 4..37 (in place), chunked
            for lo, hi in ((4, 13), (13, 22), (22, 30), (30, 38)):
                nc.scalar.activation(
                    out=h4[:, lo:hi, 5:261], in_=h4[:, lo:hi, 5:261],
                    func=mybir.ActivationFunctionType.Prelu,
                    bias=nb1, scale=rstd1, alpha=SLOPE,
                )

            # ---- layer 2 ----
            def write2(u0, pt):
                q = u0 - 4
                nc.scalar.copy(
                    out=o4[:, q:q + 2, :],
                    in_=pt[:, :].rearrange("p (a b) -> p a b", a=2),
                )

            _conv_layer(nc, pools, h4, w2t, recs2, ST2, set(ST2), write2, 4)
            rstd2, nb2 = _stats_to_scale_bias(nc, pools, recs2, cc2_in, cc2_out, eps_t)

            for lo, hi in ((0, 8), (8, 16), (16, 24), (24, 32)):
                nc.scalar.activation(
                    out=o4[:, lo:hi, :], in_=o4[:, lo:hi, :],
                    func=mybir.ActivationFunctionType.Prelu,
                    bias=nb2, scale=rstd2, alpha=SLOPE,
                )
                nc.sync.dma_start(out=out[:, lo:hi, :], in_=o4[:, lo:hi, :])

    nc.compile()
    return nc


_NC_CACHE = None


def _get_nc():
    global _NC_CACHE
    if _NC_CACHE is None:
        _NC_CACHE = build_nc()
    return _NC_CACHE


# ------------------------------------------------------------------
# host wrapper
# ------------------------------------------------------------------

def _prep_core_inputs(image, kern1_pk, kern1_pk_f, kern2_pk, kern2_pk_f):
    in_maps = []
    for core in range(8):
        b, half = core // 2, core % 2
        if half == 0:
            xs = image[b, :, 0:NROWS_IN, :]
        else:
            xs = image[b, :, H - NROWS_IN:H, :][:, ::-1, :]
        xpad = np.zeros((C, NPX * 4, W), np.float32)
        xpad[:, RHO0:RHO0 + NROWS_IN, :] = xs
        # interleave rows: partition g*32+i holds rows 4q+g
        x4 = np.ascontiguousarray(
            xpad.reshape(C, NPX, 4, W).transpose(2, 0, 1, 3).reshape(128, NPX, W)
        )
        in_maps.append({
            "xin": x4,
            "w1": kern1_pk if half == 0 else kern1_pk_f,
            "w2": kern2_pk if half == 0 else kern2_pk_f,
        })
    return in_maps


def _assemble(results):
    out = np.empty((B, C, H, W), np.float32)
    for core in range(8):
        b, half = core // 2, core % 2
        o4 = results[core]["out"]  # [128, 32, 256]
        # rows r2 = 4q + d; o4[d*32+i, q, w]
        rows = o4.reshape(4, C, NPO, W).transpose(1, 2, 0, 3).reshape(C, 128, W)
        if half == 0:
            out[b, :, 0:128, :] = rows
        else:
            out[b, :, 128:256, :] = rows[:, ::-1, :]
    return out


def kernel(image, weight1, weight2):
    image = np.asarray(image, dtype=np.float32)
    weight1 = np.asarray(weight1, dtype=np.float32)
    weight2 = np.asarray(weight2, dtype=np.float32)

    psi = _make_psi()
    kern1 = np.einsum("oik,khw->oihw", weight1, psi).astype(np.float32)
    kern2 = np.einsum("oik,khw->oihw", weight2, psi).astype(np.float32)

    k1p = pack_weights(kern1)
    k1pf = pack_weights(kern1[:, :, ::-1, :])
    k2p = pack_weights(kern2)
    k2pf = pack_weights(kern2[:, :, ::-1, :])

    nc = _get_nc()
    in_maps = _prep_core_inputs(image, k1p, k1pf, k2p, k2pf)
    res = run_bass_kernel_spmd(nc, in_maps, core_ids=list(range(8)))
    return _assemble(res.results)
